# revision 17
# baseline (speedup 1.0000x reference)
"""ACT-R recurrence kernel, v5: PWL-exp2 on DVE + PE-matmul reduce.

Layout: batch on partitions ([128 p] x [16 bc]); per core B=2048.
D state in PSUM [P, S(j), NC] f32 maintained by identity-stationary
accumulating matmuls from f16 gap vectors (baseline scheme).

Per step i (lag-1 split, S_i = P_i + F_i):
  Ln_i   (ACT): LR = f16(Ln(scale * psum_D[0:i]))          1 inst/step
  fresh  (DVE): NEG2[i] = sttA/sttB over pwl2(LR_i[i-1]*NEG2[i-1])
  bulk i+1:
    QK   (DVE): QK = LR_{i+1}[0:i] * NEG2[0:i]   f16, 2x
    U1/U2(DVE): (QK max -14724) + {15226,14724} -> i16, 4x
    red  (PE) : PR[p,bc] = sum_j bitcast_f16(U1)+bitcast_f16(U2)
                via two identity matmuls with 0-stride-out accumulation
    NP   (DVE): NP = PR * K1GK + K0K   (tiny, psum read)
NEG2 stores -decay*1024*log2e (f16); the i16 bitcast IS the exp2.
exp never touches ACT; the reduce never runs on DVE; no gpsimd/Pool.
Final: S = (NEG2 - K0)/K1E; out = Sigmoid((Ln(S) - tau)/s).
"""

import sys

for _p in ("/opt/trn_rl_repo",):
    if _p not in sys.path:
        sys.path.insert(0, _p)

import numpy as np
from contextlib import ExitStack

import concourse.hw_specs as hw_specs
import concourse.bacc as bacc_mod
from concourse import mybir

_orig_gat = hw_specs.get_activation_tables


def _patched_gat(arch):
    tabs = _orig_gat(arch)
    out = {}
    ln_t = mybir.ActivationFunctionType.Ln
    exp_t = mybir.ActivationFunctionType.Exp
    for name, funcs in tabs.items():
        f = set(funcs)
        if name != "natural_log_exp_and_others":
            f.discard(ln_t)
            f.discard(exp_t)
        out[name] = f
    return out


bacc_mod.get_activation_tables = _patched_gat

import concourse.bass as bass
import concourse.bacc as bacc
import concourse.tile as tile
from concourse.bass_utils import run_bass_kernel_spmd

S = 128
B_FULL = 16384
N_CORES = 8
B = B_FULL // N_CORES  # 2048 per core
P = 128
NC = B // P  # 16

F32 = mybir.dt.float32
F16 = mybir.dt.float16
I16 = mybir.dt.int16
AF = mybir.ActivationFunctionType
ALU = mybir.AluOpType

LOG2E = 1.4426950408889634
GAMMA = 0.6175430020692696
BIAS1 = 15226.0
BIAS2 = 14724.0
CLAMP = -14724.0

NL = 4  # LR ring slots (W-deep D/Ln lookahead)
W = 3


def build_kernel(a, c, s, tau, h, repeat=1):
    scale = 86400.0 * float(h)
    K1GK = -float(c) * 1024.0 * LOG2E * GAMMA
    K1EK = -float(c) * 1024.0 * LOG2E
    K0K = -float(a) * 1024.0 * LOG2E
    nc = bacc.Bacc()

    grt_in = nc.declare_dram_parameter("grt", [P, NC, S], F16, isOutput=False)
    eye_in = nc.declare_dram_parameter("eye", [P, P], F16, isOutput=False)
    out_ext = nc.declare_dram_parameter("out", [P, NC, S], F32, isOutput=True)

    with ExitStack() as ctx:
        tc = ctx.enter_context(tile.TileContext(nc))
        pool = ctx.enter_context(tc.tile_pool(name="p", bufs=1))

        GRT = pool.tile([P, NC, S], F16)
        nc.sync.dma_start(out=GRT[:], in_=grt_in[:])
        EYE = pool.tile([P, P], F16)
        nc.sync.dma_start(out=EYE[:], in_=eye_in[:])

        NEG2 = pool.tile([P, NC, S], F16)
        NP = pool.tile([P, 3, NC], F32)
        LR = [pool.tile([P, NC, S], F16, name=f"LR{r}") for r in range(NL)]
        QK = [pool.tile([P, 2048], F16, name=f"QK{r}") for r in range(2)]
        U1 = [pool.tile([P, 2048], I16, name=f"U1{r}") for r in range(2)]
        U2 = [pool.tile([P, 2048], I16, name=f"U2{r}") for r in range(2)]
        QF = [pool.tile([P, NC, 2], F16, name=f"QF{r}") for r in range(2)]
        UF1 = [pool.tile([P, NC, 2], I16, name=f"UF1{r}") for r in range(2)]
        UF2 = [pool.tile([P, NC, 2], I16, name=f"UF2{r}") for r in range(2)]
        TFA = [pool.tile([P, NC, 2], F16, name=f"TFA{r}") for r in range(2)]
        TFS = [pool.tile([P, NC], F16, name=f"TFS{r}") for r in range(2)]
        BIAS = pool.tile([P, 1], F32)
        nc.vector.memset(BIAS[:], -float(tau) / float(s))

        DD = ctx.enter_context(nc.psum_tensor([P, S, NC], F32))
        PR = ctx.enter_context(nc.psum_tensor([P, 3, 512], F32))

        grta = GRT[:]
        eyea = EYE[:]
        dda = DD[:]
        pra = PR[:]
        n2a = NEG2[:]
        npa = NP[:]

        def emit_inc(k):
            # psum_D[:, 0:k, :] += gap_k (j-outer prefix, 512-col chunks)
            nf = k * NC
            c0 = 0
            while c0 < nf:
                c1 = min(c0 + 512, nf)
                outap = bass.AP(dda.tensor, dda.offset + c0,
                                [dda.ap[0], [1, c1 - c0]])
                mov = bass.AP(grta.tensor, grta.offset + k,
                              [grta.ap[0], [0, (c1 - c0) // NC], [S, NC]])
                nc.tensor.matmul(outap, eyea, mov,
                                 start=False, stop=False, skip_group_check=True)
                c0 = c1

        def emit_ln(i, r):
            # LR[r][:, :, 0:i] = f16(Ln(scale * D[:, 0:i, :]))  (transposing AP)
            lra = LR[r][:]
            inap = bass.AP(dda.tensor, dda.offset,
                           [dda.ap[0], [NC, i], [1, NC]])
            outap = bass.AP(lra.tensor, lra.offset,
                            [lra.ap[0], [1, i], [S, NC]])
            nc.scalar.activation(outap, inap, AF.Ln, scale=scale)

        for _rep in range(repeat):
            nc.vector.memset(NEG2[:], float(np.float16(K0K)))
            nc.vector.memset(NP[:], K0K)
            nc.vector.memset(DD[:], 0.0)

            for k in range(1, min(W + 1, S)):
                emit_inc(k)
                emit_ln(k, k % NL)

            for i in range(1, S):
                if i + W < S:
                    emit_inc(i + W)
                    emit_ln(i + W, (i + W) % NL)
                x = i % 2
                lri = LR[i % NL][:]

                # fresh chain for step i: js {max(i-2,0) .. i-1} (2-col mini)
                na = min(2, i)
                jf = i - na
                i0 = bass.AP(lri.tensor, lri.offset + jf,
                             [lri.ap[0], [S, NC], [1, na]])
                i1 = bass.AP(n2a.tensor, n2a.offset + jf,
                             [n2a.ap[0], [S, NC], [1, na]])
                nc.vector.tensor_tensor(
                    out=bass.AP(QF[x][:].tensor, QF[x][:].offset,
                                [QF[x][:].ap[0], [2, NC], [1, na]]),
                    in0=i0, in1=i1, op=ALU.mult)
                uq = bass.AP(QF[x][:].tensor, QF[x][:].offset,
                             [QF[x][:].ap[0], [2, NC], [1, na]])
                ug1 = bass.AP(UF1[x][:].tensor, UF1[x][:].offset,
                              [UF1[x][:].ap[0], [2, NC], [1, na]])
                ug2 = bass.AP(UF2[x][:].tensor, UF2[x][:].offset,
                              [UF2[x][:].ap[0], [2, NC], [1, na]])
                nc.vector.tensor_scalar(out=ug1, in0=uq,
                                        scalar1=CLAMP, scalar2=BIAS1,
                                        op0=ALU.max, op1=ALU.add)
                nc.vector.tensor_scalar(out=ug2, in0=uq,
                                        scalar1=CLAMP, scalar2=BIAS2,
                                        op0=ALU.max, op1=ALU.add)
                # TFa = bit(U1)+bit(U2) [NC, na]; TF = col-sum -> [NC]
                tb1 = UF1[x][:].bitcast(F16)
                tb2 = UF2[x][:].bitcast(F16)
                tfa = TFA[x][:]
                nc.vector.tensor_tensor(
                    out=bass.AP(tfa.tensor, tfa.offset,
                                [tfa.ap[0], [2, NC], [1, na]]),
                    in0=bass.AP(tb1.tensor, tb1.offset,
                                [tb1.ap[0], [2, NC], [1, na]]),
                    in1=bass.AP(tb2.tensor, tb2.offset,
                                [tb2.ap[0], [2, NC], [1, na]]),
                    op=ALU.add)
                if na == 2:
                    nc.vector.tensor_tensor(
                        out=TFS[x][:],
                        in0=bass.AP(tfa.tensor, tfa.offset, [tfa.ap[0], [2, NC]]),
                        in1=bass.AP(tfa.tensor, tfa.offset + 1,
                                    [tfa.ap[0], [2, NC]]),
                        op=ALU.add)
                    tfsum = TFS[x][:]
                else:
                    tfsum = bass.AP(tfa.tensor, tfa.offset, [tfa.ap[0], [2, NC]])
                on = bass.AP(n2a.tensor, n2a.offset + i, [n2a.ap[0], [S, NC]])
                npslot = bass.AP(npa.tensor, npa.offset + (i % 3) * NC,
                                 [npa.ap[0], [1, NC]])
                nc.vector.scalar_tensor_tensor(
                    out=on, in0=tfsum, scalar=K1GK,
                    in1=npslot, op0=ALU.mult, op1=ALU.add)

                # bulk for step i+2 over j < i (2-step lag: PE off the chain)
                if i + 2 < S:
                    jb = i
                    r1 = (i + 2) % NL
                    lr1 = LR[r1][:]
                    in0 = bass.AP(lr1.tensor, lr1.offset,
                                  [lr1.ap[0], [S, NC], [1, jb]])
                    in1 = bass.AP(n2a.tensor, n2a.offset,
                                  [n2a.ap[0], [S, NC], [1, jb]])
                    outq = bass.AP(QK[x][:].tensor, QK[x][:].offset,
                                   [QK[x][:].ap[0], [jb, NC], [1, jb]])
                    nc.vector.tensor_tensor(out=outq, in0=in0, in1=in1,
                                            op=ALU.mult)
                    qf2d = bass.AP(QK[x][:].tensor, QK[x][:].offset,
                                   [QK[x][:].ap[0], [1, NC * jb]])
                    o1 = bass.AP(U1[x][:].tensor, U1[x][:].offset,
                                 [U1[x][:].ap[0], [1, NC * jb]])
                    o2 = bass.AP(U2[x][:].tensor, U2[x][:].offset,
                                 [U2[x][:].ap[0], [1, NC * jb]])
                    nc.vector.tensor_scalar(out=o1, in0=qf2d, scalar1=CLAMP,
                                            scalar2=BIAS1, op0=ALU.max,
                                            op1=ALU.add)
                    nc.vector.tensor_scalar(out=o2, in0=qf2d, scalar1=CLAMP,
                                            scalar2=BIAS2, op0=ALU.max,
                                            op1=ALU.add)
                    # PE reduce: PR[x][:, g] = sum_j (bit(U1)+bit(U2))[g*jb+j]
                    # chunked to <=512 elements per matmul (32 j per chunk)
                    b1 = o1.bitcast(F16)
                    b2 = o2.bitcast(F16)
                    first = True
                    for bsrc in (b1, b2):
                        j0 = 0
                        while j0 < jb:
                            j1 = min(j0 + 32, jb)
                            prslot = bass.AP(pra.tensor, pra.offset + (i % 3) * 512,
                                             [pra.ap[0], [0, j1 - j0], [1, NC]])
                            mov = bass.AP(bsrc.tensor, bsrc.offset + j0,
                                          [bsrc.ap[0], [1, j1 - j0], [jb, NC]])
                            nc.tensor.matmul(prslot, eyea, mov,
                                             start=first, stop=False,
                                             skip_group_check=True)
                            first = False
                            j0 = j1
                    # NP[(i+1)%3] = PR * K1GK + K0K
                    prread = bass.AP(pra.tensor, pra.offset + (i % 3) * 512,
                                     [pra.ap[0], [1, NC]])
                    npout = bass.AP(npa.tensor,
                                    npa.offset + ((i + 2) % 3) * NC,
                                    [npa.ap[0], [1, NC]])
                    nc.vector.tensor_scalar(out=npout, in0=prread,
                                            scalar1=K1GK, scalar2=K0K,
                                            op0=ALU.mult, op1=ALU.add)

        # epilogue
        SS = pool.tile([P, NC, S], F32)
        nc.vector.tensor_scalar(
            out=SS[:, :, 1:S], in0=NEG2[:, :, 1:S],
            scalar1=-K0K, scalar2=1.0 / K1EK,
            op0=ALU.add, op1=ALU.mult,
        )
        M = pool.tile([P, NC, S], F32)
        nc.scalar.activation(M[:, :, 1:S], SS[:, :, 1:S], AF.Ln)
        O = pool.tile([P, NC, S], F32)
        nc.vector.memset(O[:, :, 0:1], 0.0)
        nc.scalar.activation(
            O[:, :, 1:S], M[:, :, 1:S], AF.Sigmoid,
            scale=1.0 / float(s), bias=BIAS[:],
        )
        nc.sync.dma_start(out=out_ext[:], in_=O[:])

    nc.compile()
    return nc


def make_in_maps(sp: np.ndarray) -> list:
    eye = np.eye(P, dtype=np.float16)
    in_maps = []
    for ci in range(N_CORES):
        shard = sp[:, ci * B : (ci + 1) * B].astype(np.float32)  # [S, B]
        gaps = np.empty_like(shard)
        gaps[0] = shard[0]
        gaps[1:] = shard[1:] - shard[:-1]
        # grt[p, bc, j] = gaps[j, bc*128 + p]
        grt = np.ascontiguousarray(
            gaps.reshape(S, NC, P).transpose(2, 1, 0)
        ).astype(np.float16)
        in_maps.append({"grt": grt, "eye": eye})
    return in_maps


def kernel(sp: np.ndarray, w: np.ndarray) -> np.ndarray:
    sp = np.ascontiguousarray(sp, dtype=np.float32)
    w = np.asarray(w, dtype=np.float32)
    a, c, s, tau, h = (float(x) for x in w)

    nc = build_kernel(a, c, s, tau, h)
    in_maps = make_in_maps(sp)

    res = run_bass_kernel_spmd(nc, in_maps, core_ids=list(range(N_CORES)))
    outs = []
    for ci in range(N_CORES):
        o = res.results[ci]["out"]  # [P, NC, S]
        outs.append(o.transpose(2, 1, 0).reshape(S, B)[1:S])
    return np.concatenate(outs, axis=1).astype(np.float32)


if __name__ == "__main__":
    rng = np.random.default_rng(0)
    spt = np.cumsum(rng.uniform(0.1, 5.0, (S, B_FULL)).astype(np.float32), axis=0)
    wt = np.asarray(
        [0.176786766570677, 0.216967308403809, 0.254893976981164,
         -0.704205679427144, 0.025], dtype=np.float32)
    o = kernel(spt, wt)
    print(o.shape, o.dtype, o[:3, :3])
